# revision 3
# baseline (speedup 1.0000x reference)
"""MixerAttention (GQA + QK-RMSNorm + RoPE + causal) Trainium2 kernel.

Sharding: 8 cores = batch(2) x kv-head(4). Fully local per core — no collectives.
Per core: project its 4 q heads + 1 kv head (x^T/W^T pre-transposed on host so
the D contraction lands on partitions, f32r so the PE runs at full rate), QK
RMSNorm via the ln/exp rsqrt path + RoPE, then causal attention in S^T layout.

Structure (the PE runs at the causal-MAC roofline; everything else hides
behind it):
- Softmax denominators are free on the PE: PV uses exp(S^T) 128x128 blocks as
  the STATIONARY operand and V as the MOVING operand with a ones-column
  appended ([V | 1], bf16): each PSUM accumulation chain yields y[q, 0:128]
  and the softmax denominator at column 128, eliminating the ones-matmul
  pass (~70k PE cycles/core) a separate denominator reduction costs. y lands
  q-major, so host reassembly is a plain column-block copy.
- The RMSNorm partition sum-of-squares runs on the otherwise-idle GPSIMD
  (partition_all_reduce), the square on the DVE, and RoPE commutes with the
  normalization (rs is partition-constant) so its muls overlap the ln/exp
  round-trip on the scalar engine.
- exp is batched two k-blocks per ACT instruction ([128, 2, 512] PSUM
  supertiles); additive causal mask on diagonal blocks; the thinnest
  diagonal S matmul is widened to 256 columns to stay at the f32r full-rate
  threshold (the extra columns are finite garbage no PV chain reads).
- Per q-window, each head's chain -> S/exp is interleaved with the previous
  head's PV and the next window's projection wave so the PE never waits on
  the exp round-trip.
- Tile pools are shared across reps (no per-rep drain barriers) and weights
  are split by m-group (wtkv/wtq) so a following rep's weight DMA starts as
  soon as the previous rep's last consumer of that slice retires: reps
  software-pipeline into a steady PE-bound stream.
"""
import sys

sys.path.insert(0, "/opt/trn_rl_repo")
from contextlib import ExitStack

import numpy as np
import concourse.bacc as bacc
import concourse.mybir as mybir
import concourse.tile as tile
from concourse.bass_utils import run_bass_kernel_spmd
from concourse.bass_isa import ReduceOp
from concourse.masks import make_identity

F32 = mybir.dt.float32
F32R = mybir.dt.float32r
BF16 = mybir.dt.bfloat16
AF = mybir.ActivationFunctionType

B, T, D = 2, 2048, 2048
H, HKV, DH = 16, 4, 128
G = H // HKV                    # q heads per kv head (per core)
EPS = 1.1920928955078125e-07
ROPE_BASE = 10000.0
NCORES = 8

P = 128                         # partitions
DCH = D // P                    # 16 d-chunks (contraction)
NT = 4                          # column windows of 512
TC = T // NT                    # 512
EQ = G * DH                     # 512
ETOT = EQ + DH + DH             # 768
QC = 512                        # attention q-chunk == TC
KC = 128                        # attention k-chunk
NKC = T // KC                   # 16
NEG = -1.0e30
MK, MV = G, G + 1               # m-tile indices of k and v rows
VW = 130                        # Vaug row: 128 v + 1 ones + 1 pad


def _chain(nc, pools, src, dst, dst0, ln_scale, ln_bias, ropeC, ropeS, n, label):
    """Per-512-chunk RMSNorm (ln/exp rsqrt) + RoPE: src (P,TC) fp32 staging
    -> dst[:, dst0:dst0+TC] f32r.

    rs (the rsqrt of the mean square) is identical on every partition, so
    RoPE commutes with the normalization: dst = (x*C + rot(x)*S) * rs. The
    RoPE muls run on the DVE in parallel with the ln/exp round-trip instead
    of serially after it, the square runs on the DVE to keep the ACT queue
    free for attention exp, and the partition sum of squares runs on the
    otherwise-idle GPSIMD so it costs no PE cycles."""
    sp, cps = pools
    c0 = n * TC
    sq = sp.tile([P, TC], F32, tag="sq", name=f"sq_{label}")
    nc.vector.tensor_mul(sq, src, src)
    ssb = sp.tile([P, TC], F32, tag="ssb", name=f"ssbt_{label}")
    nc.gpsimd.partition_all_reduce(ssb, sq, channels=P, reduce_op=ReduceOp.add)
    lnt = sp.tile([P, TC], F32, tag="lnt", name=f"lnt_{label}")
    nc.scalar.activation(lnt, ssb, AF.Ln, scale=ln_scale, bias=ln_bias[:, :])
    rs = sp.tile([P, TC], F32, tag="rs", name=f"rs_{label}")
    nc.scalar.activation(rs, lnt, AF.Exp, scale=-0.5)
    # rope on the un-normalized src, in parallel with ln/exp
    tmp = sp.tile([P, TC], F32, tag="rtmp", name=f"rtmp_{label}")
    nc.vector.tensor_copy(tmp[0 : P // 2, :], src[P // 2 : P, :])
    nc.vector.tensor_copy(tmp[P // 2 : P, :], src[0 : P // 2, :])
    t1 = sp.tile([P, TC], F32, tag="rt1", name=f"rt1_{label}")
    nc.vector.tensor_mul(t1, src, ropeC[:, c0 : c0 + TC])
    nc.vector.tensor_mul(tmp, tmp, ropeS[:, c0 : c0 + TC])
    nc.vector.tensor_add(t1, t1, tmp)
    nc.vector.tensor_mul(dst[:, dst0 : dst0 + TC], t1, rs)


def _make_pools(tc, ctx):
    return {
        "constp": ctx.enter_context(tc.tile_pool(name="const", bufs=1)),
        "finp": ctx.enter_context(tc.tile_pool(name="final", bufs=1)),
        "wp": ctx.enter_context(tc.tile_pool(name="wp", bufs=1)),
        "xp": ctx.enter_context(tc.tile_pool(name="xp", bufs=19)),
        "stg": ctx.enter_context(tc.tile_pool(name="stg", bufs=9)),
        "sp": ctx.enter_context(tc.tile_pool(name="sp", bufs=2)),
        "qsc": ctx.enter_context(tc.tile_pool(name="qsc", bufs=8)),
        "esp": ctx.enter_context(tc.tile_pool(name="esp", bufs=16)),
        "yop": ctx.enter_context(tc.tile_pool(name="yop", bufs=3)),
        "recp": ctx.enter_context(tc.tile_pool(name="recp", bufs=3)),
        "cps": ctx.enter_context(tc.tile_pool(name="cps", bufs=1, space="PSUM")),
    }


def _body(nc, tc, pools, rep):
    XT = nc.cur_io["xT"]
    WT = nc.cur_io["wT"]
    RC = nc.cur_io["ropeC"]
    RS_ = nc.cur_io["ropeS"]
    TRI = nc.cur_io["trineg"]
    YT = nc.cur_io["yT"]

    constp, finp, wp, xp, stg, sp, qsc, esp, yop, recp, cps = (
        pools["constp"], pools["finp"], pools["wp"], pools["xp"], pools["stg"],
        pools["sp"], pools["qsc"], pools["esp"], pools["yop"], pools["recp"],
        pools["cps"])
    R = f"r{rep}"

    # weights split by m-group (kv cols / q cols) so the next rep's weight
    # DMA can start as soon as this rep's last consumer of that slice is done;
    # x loads in 4-d-chunk supertiles (1 MiB DMAs, few descriptors)
    wtkv = wp.tile([P, DCH, 2 * DH], F32R, tag="wtkv", name=f"wtkv_{R}")
    wtq = wp.tile([P, DCH, EQ], F32R, tag="wtq", name=f"wtq_{R}")

    def wslice(m, d0=None, d1=None):
        if m >= MK:
            return wtkv[:, d0:d1, :] if d0 is not None else wtkv
        return wtq[:, d0:d1, :] if d0 is not None else wtq

    def wcol(m, d):
        if m >= MK:
            return wtkv[:, d, (m - MK) * P : (m - MK + 1) * P]
        return wtq[:, d, m * P : (m + 1) * P]

    xns = {}

    def load_x(n, d):
        xn = xp.tile([P, TC], F32R, tag="xn", name=f"xn_{n}_{d}_{R}")
        nc.sync.dma_start(out=xn, in_=XT[d, :, n * TC : (n + 1) * TC])
        xns[(n, d)] = xn

    wgroups = [(0, 4), (4, 8), (8, 12), (12, 16)]
    for lo, hi in wgroups:
        nc.sync.dma_start(
            out=wtkv[:, lo:hi, :],
            in_=WT[lo:hi, :, EQ:ETOT].rearrange("d p e -> p d e"),
        )
        for d in range(lo, hi):
            load_x(0, d)
    for lo, hi in wgroups:
        nc.sync.dma_start(
            out=wtq[:, lo:hi, :],
            in_=WT[lo:hi, :, 0:EQ].rearrange("d p e -> p d e"),
        )

    trineg = constp.tile([P, KC], F32, tag="trineg", name=f"trineg_{R}")
    nc.sync.dma_start(out=trineg, in_=TRI[:, :])
    ropeC = constp.tile([P, T], F32, tag="ropeC", name=f"ropeC_{R}")
    nc.sync.dma_start(out=ropeC, in_=RC[:, :])
    ropeS = constp.tile([P, T], F32, tag="ropeS", name=f"ropeS_{R}")
    nc.sync.dma_start(out=ropeS, in_=RS_[:, :])
    identf = constp.tile([P, P], F32, tag="identf", name=f"identf_{R}")
    make_identity(nc, identf)
    ident = constp.tile([P, P], F32R, tag="ident", name=f"ident_{R}")
    nc.vector.tensor_copy(ident, identf)
    bq = constp.tile([P, 1], F32, tag="bq", name=f"bq_{R}")
    nc.vector.memset(bq, float(P) * EPS)
    bk = constp.tile([P, 1], F32, tag="bk", name=f"bk_{R}")
    nc.vector.memset(bk, EPS)

    KTr = finp.tile([P, T], F32R, tag="KTr", name=f"KTr_{R}")
    Vaug = finp.tile([P, NKC, VW], BF16, tag="Vaug", name=f"Vaug_{R}")
    nc.vector.memset(Vaug[:, :, P : P + 1], 1.0)   # ones column for denominators

    def s_exp(g, i, qtr):
        """S^T blocks for head g, q-window i: matmul into [P,2,QC] PSUM pairs,
        causal mask on diagonal, exp -> bf16 es pair tiles. Returns es tiles."""
        nk = 4 * (i + 1)
        es_tiles = []
        for jp in range(nk // 2):
            sps = cps.tile([P, 2, QC], F32, tag="sps", bufs=2, name=f"sps_{g}_{i}_{jp}_{R}")
            es = esp.tile([P, 2, QC], BF16, tag="es", name=f"es_{g}_{i}_{jp}_{R}")
            diag = jp >= 2 * i
            for jj in range(2):
                j = 2 * jp + jj
                dcol = max(0, j * KC - i * QC)
                # keep fp32r moving width >= 256 (narrower runs at 1/4 rate);
                # the extra [256:384) columns of the last diagonal block are
                # finite garbage that no PV chain reads
                mcol = min(dcol, QC - 2 * KC)
                nc.tensor.matmul(
                    sps[:, jj, mcol:QC],
                    KTr[:, j * KC : (j + 1) * KC],
                    qtr[:, mcol:QC],
                    start=True,
                    stop=True,
                )
                if j * KC >= i * QC:  # diagonal: additive causal mask
                    nc.vector.tensor_add(
                        sps[:, jj, dcol : dcol + KC],
                        sps[:, jj, dcol : dcol + KC],
                        trineg,
                    )
            if jp == 2 * i + 1:
                # second diagonal pair: both blocks were computed on
                # [2*KC:QC] (widened), one exp covers them
                nc.scalar.activation(
                    es[:, :, 2 * KC : QC], sps[:, :, 2 * KC : QC], AF.Exp
                )
            elif diag:
                for jj in range(2):
                    j = 2 * jp + jj
                    dcol = max(0, j * KC - i * QC)
                    nc.scalar.activation(
                        es[:, jj, dcol:QC], sps[:, jj, dcol:QC], AF.Exp
                    )
            else:
                nc.scalar.activation(es, sps, AF.Exp)
            es_tiles.append(es)
        return es_tiles

    def pv(g, i, es_tiles):
        """Per 128-query block: accumulate [y | denom] = es^T-stationary @
        [V | 1] over k-blocks; normalize on DVE; DMA out q-major."""
        for qs in range(4):
            nj = 4 * i + qs + 1
            yps = cps.tile([P, QC], F32, tag="yps", bufs=2, name=f"yps_{g}_{i}_{qs}_{R}")
            for j in range(nj):
                jp, jj = divmod(j, 2)
                nc.tensor.matmul(
                    yps[:, 0 : P + 1],
                    es_tiles[jp][:, jj, qs * KC : (qs + 1) * KC],
                    Vaug[:, j, 0 : P + 1],
                    start=(j == 0),
                    stop=(j == nj - 1),
                )
            rec = recp.tile([P, 1], F32, tag="rec", name=f"rec_{g}_{i}_{qs}_{R}")
            nc.vector.reciprocal_approx_fast(out=rec, in_=yps[:, P : P + 1])
            yo = yop.tile([P, KC], F32, tag="yo", name=f"yo_{g}_{i}_{qs}_{R}")
            nc.vector.tensor_scalar_mul(yo, yps[:, 0:P], rec)
            nc.sync.dma_start(
                out=YT[
                    i * QC + qs * KC : i * QC + (qs + 1) * KC,
                    g * DH : (g + 1) * DH,
                ],
                in_=yo,
            )

    WAVES = ([MK, MV], [0, 1], [2, 3])

    def proj_wave(n, wi, stage):
        wave = WAVES[wi]
        psl = {
            m: cps.tile([P, TC], F32, tag="pj", bufs=2, name=f"pj_{n}_{m}_{R}")
            for m in wave
        }
        for d in range(DCH):
            for m in wave:
                nc.tensor.matmul(
                    psl[m],
                    wcol(m, d),
                    xns[(n, d)],
                    start=(d == 0),
                    stop=(d == DCH - 1),
                )
        for m in wave:
            dt = F32R if m == MV else F32
            st = stg.tile([P, TC], dt, tag="stage", name=f"st_{n}_{m}_{R}")
            nc.scalar.copy(st, psl[m])
            stage[m] = st

    stages = {n: {} for n in range(NT)}
    for wi in range(3):
        proj_wave(0, wi, stages[0])

    for n in range(NT):
        # prefetch next window's x
        if n + 1 < NT:
            for d in range(DCH):
                load_x(n + 1, d)
        stage = stages[n]

        # ---- chains: k first, then V transposes ----
        _chain(nc, (sp, cps), stage[MK], KTr, n * TC, 1.0 / P, bk,
               ropeC, ropeS, n, f"k{n}_{R}")
        for jj in range(4):
            j = 4 * n + jj
            vps = cps.tile([P, QC], F32R, tag="yps", bufs=2, name=f"vps_{j}_{R}")
            nc.tensor.transpose(
                vps[:, 0:KC], stage[MV][:, jj * KC : (jj + 1) * KC], ident
            )
            nc.vector.tensor_copy(Vaug[:, j, 0:P], vps[:, 0:KC])

        # ---- per head: chain -> S/exp; next window's projection wave and
        # the previous head's PV fill the PE while this head's exp drains ----
        i = n
        prev = None
        for g in range(G):
            qt = qsc.tile([P, TC], F32R, tag="qtr", name=f"qtr_{g}_{n}_{R}")
            _chain(nc, (sp, cps), stage[g], qt, 0, 1.0, bq,
                   ropeC, ropeS, n, f"q{g}_{n}_{R}")
            es_tiles = s_exp(g, i, qt)
            if n + 1 < NT and g < 3:
                proj_wave(n + 1, g, stages[n + 1])
            if prev is not None:
                pv(g - 1, i, prev)
            prev = es_tiles
        pv(G - 1, i, prev)


def _pin_act_table_set():
    """Restrict the ACT table chooser to natural_log_exp_and_others (which
    holds ln/exp/square/copy — every function this kernel uses) so the
    compiled stream has one table load instead of one per ln<->exp switch
    (~1.3us each). Indices of the full set list are preserved."""
    import concourse.hw_specs as hw_specs

    if getattr(bacc, "_act_tables_pinned", False):
        return
    orig = hw_specs.get_activation_tables
    keep = "natural_log_exp_and_others"

    def patched(arch):
        t = orig(arch)
        return {k: (v if k == keep else set()) for k, v in t.items()}

    bacc.get_activation_tables = patched
    bacc._act_tables_pinned = True


def build_nc(reps=1):
    _pin_act_table_set()
    nc = bacc.Bacc(trn_type="TRN2")
    nc.cur_io = {
        "xT": nc.dram_tensor("xT", [DCH, P, T], F32R, kind="ExternalInput"),
        "wT": nc.dram_tensor("wT", [DCH, P, ETOT], F32R, kind="ExternalInput"),
        "ropeC": nc.dram_tensor("ropeC", [P, T], F32, kind="ExternalInput"),
        "ropeS": nc.dram_tensor("ropeS", [P, T], F32, kind="ExternalInput"),
        "trineg": nc.dram_tensor("trineg", [P, KC], F32, kind="ExternalInput"),
        "yT": nc.dram_tensor("yT", [T, EQ], F32, kind="ExternalOutput"),
    }
    with tile.TileContext(nc) as tc:
        with ExitStack() as ctx:
            pools = _make_pools(tc, ctx)
            for _rep in range(reps):
                _body(nc, tc, pools, _rep)
    nc.finalize()
    return nc


_NC_CACHE = None


def _get_nc():
    global _NC_CACHE
    if _NC_CACHE is None:
        _NC_CACHE = build_nc()
    return _NC_CACHE


def _host_tables():
    inv_freq = 1.0 / (ROPE_BASE ** (np.arange(0, DH, 2, dtype=np.float32) / DH))
    t = np.arange(T, dtype=np.float32)
    freqs = np.outer(t, inv_freq).astype(np.float32)    # (T, 64)
    cosT = np.cos(freqs).T.astype(np.float32)            # (64, T)
    sinT = np.sin(freqs).T.astype(np.float32)
    ropeC = np.concatenate([cosT, cosT], axis=0)         # (128, T)
    ropeS = np.concatenate([sinT, -sinT], axis=0)
    pp_ = np.arange(KC)[:, None]
    ff = np.arange(KC)[None, :]
    trineg = np.where(pp_ <= ff, 0.0, NEG).astype(np.float32)
    return np.ascontiguousarray(ropeC), np.ascontiguousarray(ropeS), trineg


def kernel(x, Wq, Wk, Wv):
    x = np.asarray(x, dtype=np.float32)
    Wq = np.asarray(Wq, dtype=np.float32)
    Wk = np.asarray(Wk, dtype=np.float32)
    Wv = np.asarray(Wv, dtype=np.float32)
    ropeC, ropeS, trineg = _host_tables()

    in_maps = []
    for core in range(NCORES):
        b, h = divmod(core, HKV)
        xT = np.ascontiguousarray(x[b].T).reshape(DCH, P, T)
        Wsl = np.concatenate(
            [
                Wq[h * EQ : (h + 1) * EQ],
                Wk[h * DH : (h + 1) * DH],
                Wv[h * DH : (h + 1) * DH],
            ],
            axis=0,
        )                                                 # (768, D)
        wT = np.ascontiguousarray(Wsl.T).reshape(DCH, P, ETOT)
        in_maps.append(
            {"xT": xT, "wT": wT, "ropeC": ropeC, "ropeS": ropeS, "trineg": trineg}
        )

    nc = _get_nc()
    res = run_bass_kernel_spmd(nc, in_maps, core_ids=list(range(NCORES)))

    out = np.empty((B, T, H * DH), dtype=np.float32)
    for core in range(NCORES):
        b, h = divmod(core, HKV)
        out[b, :, h * EQ : (h + 1) * EQ] = res.results[core]["yT"]   # (T, 512)
    return out
